# revision 5
# baseline (speedup 1.0000x reference)
"""Block-diagonal 2x2 equalizer kernel for Trainium2 (8 NeuronCores), v2.1.

Per point (b, u, s, f) solves the 2x2 system M x = v by Cramer's rule:
    det = m00*m11 - m01*m10;  x0 = (m11*v0 - m01*v1)/det
                              x1 = (m00*v1 - m10*v0)/det

Numerics (validated on host, rel 3.3e-4 vs gate 2e-2): dets get as small as
1.5e-4 with |p| ~ 10, so the det path (m, p0, p1, det) stays f32 end to end.
Everything else is fp16: y ships fp16, the numerator runs fp16 on DVE at
2x_1P, rdet and x are fp16. HBM traffic 7.34 -> 5.5 MB/core.

Sharding: data parallel over batch, 2 batches per core. Layout per core:
  partition p = b_local*64 + sf//448, column c = u*448 + sf%448  (1792 cols)
  ha = [m11|m00] f32, hb = [m01|m10] f32, yb = [v0|v1] fp16, column-chunked.

Measured v2 (33.9us): DVE saturated; critical path = big chunk's hb landing
+ all remaining DVE work + store tail. v2.1 therefore:
  - loads per chunk ordered ha, y, hb (last chunk: ha, hb, y) and DVE order
    p0, Q, p1, det, R, RR so only p1/det/R/RR sit behind the hb gate and
    only Q/RR-chain behind the final y gate
  - X = RR.*[rdet|rdet] runs on GPSIMD (contention experiment; DVE -2.3us)
  - chunk widths [640, 896, 256]: small last chunk = short tail chain
Engines: DVE 6 TT ops/chunk; ACT cvtA/cvtB (f32->fp16), recip written twice
([rdet|rdet] contiguous keeps X in 2x mode), dummy recip preloads the ACT
table at t~7.5us; ACT also triggers stores; sync triggers loads.

Raw Bass: all waits are standalone wait_ge (walrus allows one sync per
instruction); every SBUF buffer written exactly once; same-engine RAW relies
on program order (+ DVE DRAIN), semaphores only guard cross-engine edges.
"""

from contextlib import ExitStack

import numpy as np

import concourse.bass as bass
import concourse.mybir as mybir
from concourse.bass_utils import run_bass_kernel_spmd

B, U, A, NTX, T, S, F = 16, 4, 2, 1, 8, 14, 2048
SF = S * F
NCORES = 8
BPC = B // NCORES
QW = 448
ROWS = SF // QW          # 64
COLS = U * QW            # 1792
WIDTHS = [640, 896, 256]
NCH = len(WIDTHS)
OFFS = [sum(WIDTHS[:k]) for k in range(NCH)]

TRACE = False
LAST_RESULTS = None


def _to_cols(d):
    d = d.reshape(BPC, U, ROWS, QW).transpose(0, 2, 1, 3)
    return np.ascontiguousarray(d).reshape(BPC * ROWS, COLS)


def _from_cols(m):
    d = m.reshape(BPC, ROWS, U, QW).transpose(0, 2, 1, 3)
    return np.ascontiguousarray(d).reshape(BPC, U, SF)


def _build_nc():
    f32 = mybir.dt.float32
    f16 = mybir.dt.float16
    nc = bass.Bass("TRN2")

    dha = [nc.dram_tensor(f"ha{k}", [128, 2, W], f32, kind="ExternalInput")
           for k, W in enumerate(WIDTHS)]
    dhb = [nc.dram_tensor(f"hb{k}", [128, 2, W], f32, kind="ExternalInput")
           for k, W in enumerate(WIDTHS)]
    dyb = [nc.dram_tensor(f"yb{k}", [128, 2, W], f16, kind="ExternalInput")
           for k, W in enumerate(WIDTHS)]
    dx = [nc.dram_tensor(f"xout{k}", [128, 2, W], f16, kind="ExternalOutput")
          for k, W in enumerate(WIDTHS)]

    with ExitStack() as ctx:
        sb = lambda n, shp, dt: ctx.enter_context(nc.sbuf_tensor(n, shp, dt))
        tHa = [sb(f"tHa{k}", [128, 2, W], f32) for k, W in enumerate(WIDTHS)]
        tHb = [sb(f"tHb{k}", [128, 2, W], f32) for k, W in enumerate(WIDTHS)]
        tY = [sb(f"tY{k}", [128, 2, W], f16) for k, W in enumerate(WIDTHS)]
        cA = [sb(f"cA{k}", [128, 2, W], f16) for k, W in enumerate(WIDTHS)]
        cB = [sb(f"cB{k}", [128, 2, W], f16) for k, W in enumerate(WIDTHS)]
        tp0 = [sb(f"p0_{k}", [128, W], f32) for k, W in enumerate(WIDTHS)]
        tp1 = [sb(f"p1_{k}", [128, W], f32) for k, W in enumerate(WIDTHS)]
        tdet = [sb(f"det{k}", [128, W], f32) for k, W in enumerate(WIDTHS)]
        trd = [sb(f"rd{k}", [128, 2, W], f16) for k, W in enumerate(WIDTHS)]
        tQ = [sb(f"Q{k}", [128, 2, W], f16) for k, W in enumerate(WIDTHS)]
        tR = [sb(f"R{k}", [128, 2, W], f16) for k, W in enumerate(WIDTHS)]
        tRR = [sb(f"RR{k}", [128, 2, W], f16) for k, W in enumerate(WIDTHS)]
        tX = [sb(f"X{k}", [128, 2, W], f16) for k, W in enumerate(WIDTHS)]
        scr_in = sb("scr_in", [128, 8], f32)
        scr_out = sb("scr_out", [128, 8], f32)

        semHa = [ctx.enter_context(nc.semaphore(f"semHa{k}")) for k in range(NCH)]
        semHb = [ctx.enter_context(nc.semaphore(f"semHb{k}")) for k in range(NCH)]
        semY = [ctx.enter_context(nc.semaphore(f"semY{k}")) for k in range(NCH)]
        semO = [ctx.enter_context(nc.semaphore(f"semO{k}")) for k in range(NCH)]
        dve_sem = ctx.enter_context(nc.semaphore("dve_sem"))
        act_sem = ctx.enter_context(nc.semaphore("act_sem"))
        gp_sem = ctx.enter_context(nc.semaphore("gp_sem"))

        # Precomputed 1-based sem targets.
        # DVE/chunk: p0, Q, p1, det, R, RR (6 ops)
        dve_det = [6 * k + 4 for k in range(NCH)]
        dve_RR = [6 * k + 6 for k in range(NCH)]
        # ACT/chunk: cvtA, cvtB, recipA, recipB (stores don't inc act_sem)
        act_cvtA = [4 * k + 1 for k in range(NCH)]
        act_cvtB = [4 * k + 2 for k in range(NCH)]
        act_recipB = [4 * k + 4 for k in range(NCH)]
        # GPSIMD: X(k) is op k+1
        gp_X = [k + 1 for k in range(NCH)]
        dve_i = {"n": 0}

        def recip(scalar, in_ap, out_ap):
            return scalar.add_instruction(
                mybir.InstActivation(
                    name=nc.get_next_instruction_name(),
                    func=mybir.ActivationFunctionType.Reciprocal,
                    ins=[
                        scalar.lower_ap(in_ap),
                        mybir.ImmediateValue(dtype=mybir.dt.float32, value=0.0),
                        mybir.ImmediateValue(dtype=mybir.dt.float32, value=1.0),
                        mybir.ImmediateValue(dtype=mybir.dt.float32, value=0.0),
                    ],
                    outs=[scalar.lower_ap(out_ap)],
                )
            )

        with nc.Block() as block:

            @block.sync
            def _(sync):
                for k in range(NCH):
                    sync.dma_start(out=tHa[k][:], in_=dha[k][:]).then_inc(semHa[k], 16)
                    if k < NCH - 1:
                        sync.dma_start(out=tY[k][:], in_=dyb[k][:]).then_inc(semY[k], 16)
                        sync.dma_start(out=tHb[k][:], in_=dhb[k][:]).then_inc(semHb[k], 16)
                    else:
                        sync.dma_start(out=tHb[k][:], in_=dhb[k][:]).then_inc(semHb[k], 16)
                        sync.dma_start(out=tY[k][:], in_=dyb[k][:]).then_inc(semY[k], 16)

            @block.vector
            def _(vector):
                def op(f, *a):
                    f(*a).then_inc(dve_sem, 1)
                    dve_i["n"] += 1
                    return dve_i["n"]

                for k in range(NCH):
                    vector.wait_ge(semHa[k], 16)
                    op(vector.tensor_mul, tp0[k][:], tHa[k][:, 0], tHa[k][:, 1])
                    vector.wait_ge(semY[k], 16)
                    vector.wait_ge(act_sem, act_cvtA[k])
                    op(vector.tensor_mul, tQ[k][:], cA[k][:], tY[k][:])
                    vector.wait_ge(semHb[k], 16)
                    op(vector.tensor_mul, tp1[k][:], tHb[k][:, 0], tHb[k][:, 1])
                    n = op(vector.tensor_sub, tdet[k][:], tp0[k][:], tp1[k][:])
                    assert n == dve_det[k]
                    vector.wait_ge(act_sem, act_cvtB[k])
                    op(vector.tensor_mul, tR[k][:], cB[k][:], tY[k][:, ::-1, :])
                    n = op(vector.tensor_sub, tRR[k][:], tQ[k][:], tR[k][:])
                    assert n == dve_RR[k]

            @block.gpsimd
            def _(gpsimd):
                for k in range(NCH):
                    gpsimd.wait_ge(dve_sem, dve_RR[k])
                    gpsimd.wait_ge(act_sem, act_recipB[k])
                    gpsimd.tensor_mul(tX[k][:], tRR[k][:], trd[k][:]).then_inc(gp_sem, 1)

            @block.scalar
            def _(scalar):
                nact = {"n": 0}

                def aop(inst):
                    inst.then_inc(act_sem, 1)
                    nact["n"] += 1
                    return nact["n"]

                # preload the activation table set while DMA streams in
                recip(scalar, scr_in[:], scr_out[:])

                for k in range(NCH):
                    scalar.wait_ge(semHa[k], 16)
                    n = aop(scalar.copy(cA[k][:], tHa[k][:]))
                    assert n == act_cvtA[k]
                    scalar.wait_ge(semHb[k], 16)
                    n = aop(scalar.copy(cB[k][:], tHb[k][:]))
                    assert n == act_cvtB[k]
                    if k > 0:
                        scalar.wait_ge(gp_sem, gp_X[k - 1])
                        scalar.dma_start(out=dx[k - 1][:], in_=tX[k - 1][:]).then_inc(
                            semO[k - 1], 16
                        )
                    scalar.wait_ge(dve_sem, dve_det[k])
                    aop(recip(scalar, tdet[k][:], trd[k][:, 0]))
                    n = aop(recip(scalar, tdet[k][:], trd[k][:, 1]))
                    assert n == act_recipB[k]
                last = NCH - 1
                scalar.wait_ge(gp_sem, gp_X[last])
                scalar.dma_start(out=dx[last][:], in_=tX[last][:]).then_inc(
                    semO[last], 16
                )
                for k in range(NCH):
                    scalar.wait_ge(semO[k], 16)

    return nc


def make_in_maps(y, h, precoding_ind):
    """Host-side gather + pack (indexing/layout only; no arithmetic)."""
    y = np.asarray(y)
    h = np.asarray(h)
    pi = np.asarray(precoding_ind).astype(np.int64)

    hg = h[:, pi[0]]                                     # [B, U, A, NTX, T, S, F]
    hsel = np.stack(
        [hg[:, u, :, 0, 2 * u:2 * u + 2] for u in range(U)], axis=1
    )                                                    # [B, U, 2(i), 2(j), S, F]
    hsel = np.ascontiguousarray(hsel).reshape(B, U, 4, SF).astype(np.float32)
    yr = np.ascontiguousarray(y).reshape(B, U, A, SF)

    in_maps = []
    for c in range(NCORES):
        b0 = c * BPC
        hs = hsel[b0:b0 + BPC]
        ys = yr[b0:b0 + BPC]
        m00 = _to_cols(hs[:, :, 0])
        m01 = _to_cols(hs[:, :, 1])
        m10 = _to_cols(hs[:, :, 2])
        m11 = _to_cols(hs[:, :, 3])
        v0 = _to_cols(ys[:, :, 0]).astype(np.float16)
        v1 = _to_cols(ys[:, :, 1]).astype(np.float16)
        mp = {}
        for k, (o, W) in enumerate(zip(OFFS, WIDTHS)):
            mp[f"ha{k}"] = np.ascontiguousarray(
                np.stack([m11[:, o:o + W], m00[:, o:o + W]], axis=1))
            mp[f"hb{k}"] = np.ascontiguousarray(
                np.stack([m01[:, o:o + W], m10[:, o:o + W]], axis=1))
            mp[f"yb{k}"] = np.ascontiguousarray(
                np.stack([v0[:, o:o + W], v1[:, o:o + W]], axis=1))
        in_maps.append(mp)
    return in_maps


def assemble_output(results):
    out = np.empty((B, U, A, S, F), np.float32)
    for c in range(NCORES):
        x0 = np.empty((128, COLS), np.float32)
        x1 = np.empty((128, COLS), np.float32)
        for k, (o, W) in enumerate(zip(OFFS, WIDTHS)):
            xo = np.asarray(results[c][f"xout{k}"]).astype(np.float32)
            x0[:, o:o + W] = xo[:, 0]
            x1[:, o:o + W] = xo[:, 1]
        b0 = c * BPC
        out[b0:b0 + BPC, :, 0] = _from_cols(x0).reshape(BPC, U, S, F)
        out[b0:b0 + BPC, :, 1] = _from_cols(x1).reshape(BPC, U, S, F)
    return out


def kernel(y, h, precoding_ind):
    global LAST_RESULTS
    in_maps = make_in_maps(y, h, precoding_ind)
    nc = _build_nc()
    res = run_bass_kernel_spmd(nc, in_maps, list(range(NCORES)), trace=TRACE)
    LAST_RESULTS = res
    return assemble_output(res.results)
